# revision 1
# baseline (speedup 1.0000x reference)
"""Trainium2 Bass kernel for nn_MemoryBlock (scatter_memory).

Mathematical identity: softmax over the memory-unit axis U produces rows
that sum to exactly 1, so

    out[b] = relu( mean_u( sum_n attn[b,n,u] * V[b,n,:] ) @ Wo + bo )
           = relu( ((sum_n X[b,n,:]) @ Wv + N*bv) / U @ Wo + bo )

-- the whole K/scores/softmax path cancels algebraically, leaving a
memory-bound column-sum of X (reading 134 MB is the roofline) plus two
tiny matmuls.

Sharding: data-parallel over batch B=16 across 8 cores (2 batches/core),
small weights replicated (host pre-folds Wv/U and bv*N/U).

Raw Bass (no TileContext): explicit per-engine programs and semaphores
for a minimal start/stop bracket.

Per core (2 batches): SP issues the 16x 1MB X-chunk DMAs immediately;
ACT ring carries the small constant DMAs; PE runs the fp32r ones-matmul
column-sum per chunk accumulating in PSUM, then folds/transposes; DVE/ACT
do the tiny finale; teardown is one wait on the output DMA + sem clears.
"""

import contextlib

import numpy as np

B, N, FEAT, MEM, U = 16, 8192, 256, 128, 512
NCORES = 8
BPC = B // NCORES

CH = 8              # rows-per-partition per DMA chunk -> [128, CH*FEAT] = 1 MB
RPP = N // 128      # 64 rows per partition per batch
NCH = RPP // CH     # 8 chunks per batch
MMW = 512           # matmul moving free width (one PSUM bank of fp32)
MM_PER_CHUNK = CH * FEAT // MMW  # 4

_built = None


def _ensure_axon_hooks():
    try:
        import antenv.axon_hooks  # noqa: F401
        return
    except ImportError:
        pass
    import sys
    import types

    m = types.ModuleType("antenv.axon_hooks")
    holder = [None]
    m.set_axon_ntff_profile_hook = lambda h: holder.__setitem__(0, h)
    m.get_axon_ntff_profile_hook = lambda: holder[0]
    sys.modules["antenv.axon_hooks"] = m
    try:
        import antenv

        antenv.axon_hooks = m
    except ImportError:
        pass


def _build():
    import concourse.bacc as bacc
    import concourse.mybir as mybir

    f32 = mybir.dt.float32
    f32r = mybir.dt.float32r
    AF = mybir.ActivationFunctionType
    nc = bacc.Bacc(None, enable_partition_id=False, monotonic_sem_count=0)

    X_d = nc.dram_tensor("Xs", [BPC, N, FEAT], f32r, kind="ExternalInput")
    Wv_d = nc.dram_tensor("Wvs", [2, 128, MEM], f32, kind="ExternalInput")
    Wo_d = nc.dram_tensor("Wos", [MEM, MEM], f32, kind="ExternalInput")
    # biases packed+padded to 512B/partition rows so the DMA uses
    # line-rate descriptors instead of 4-byte packets: col0=bv', col1=bo
    bias_d = nc.dram_tensor("biasc", [MEM, 128], f32, kind="ExternalInput")
    ones_d = nc.dram_tensor("onesc", [128, 128], f32r, kind="ExternalInput")
    out_d = nc.dram_tensor("outT", [MEM, BPC], f32, kind="ExternalOutput")

    ctx = contextlib.ExitStack()
    with ctx:
        xts = [
            ctx.enter_context(
                nc.sbuf_tensor(f"xt{i}", [128, CH * FEAT], f32r)
            )
            for i in range(BPC * NCH)
        ]
        ones = ctx.enter_context(nc.sbuf_tensor("ones", [128, 128], f32r))
        one_f = ctx.enter_context(nc.sbuf_tensor("one_f", [1, 1], f32))
        wv_sb = ctx.enter_context(nc.sbuf_tensor("wv_sb", [128, 2 * MEM], f32))
        wo_sb = ctx.enter_context(nc.sbuf_tensor("wo_sb", [128, MEM], f32))
        bias_sb = ctx.enter_context(nc.sbuf_tensor("bias_sb", [128, 128], f32))
        stq = ctx.enter_context(nc.sbuf_tensor("stq", [128, 2 * BPC], f32))
        srows = [
            ctx.enter_context(nc.sbuf_tensor(f"srow{i}", [1, MMW], f32))
            for i in range(BPC)
        ]
        out0 = ctx.enter_context(nc.sbuf_tensor("out0", [128, BPC], f32))
        res = ctx.enter_context(nc.sbuf_tensor("res", [128, BPC], f32))

        pss = [
            ctx.enter_context(nc.psum_tensor(f"ps{i}", [1, MMW], f32))
            for i in range(BPC)
        ]
        pts = [
            ctx.enter_context(nc.psum_tensor(f"pt{i}", [128, BPC], f32))
            for i in range(BPC)
        ]
        psv = ctx.enter_context(nc.psum_tensor("psv", [128, BPC], f32))
        pso = ctx.enter_context(nc.psum_tensor("pso", [128, BPC], f32))

        dsems = [
            ctx.enter_context(nc.semaphore(f"dsem{i}"))   # one per X chunk
            for i in range(BPC * NCH)
        ]
        csem = ctx.enter_context(nc.semaphore("csem"))    # const DMAs
        onesem = ctx.enter_context(nc.semaphore("onesem"))  # ones DMA
        osem = ctx.enter_context(nc.semaphore("osem"))    # output DMA
        pesem = ctx.enter_context(nc.semaphore("pesem"))  # PE milestones
        asem = ctx.enter_context(nc.semaphore("asem"))    # ACT milestones
        vsem = ctx.enter_context(nc.semaphore("vsem"))    # DVE milestones
        sem_nums = sorted(
            s.num for s in (*dsems, csem, onesem, osem, pesem, asem, vsem)
        )

        with nc.Block() as block:

            @block.sync
            def _(sync):
                # X chunk DMAs immediately, in consumption order (FIFO ring
                # -> in-order completion -> dsem thresholds are per-chunk)
                for b in range(BPC):
                    Xb = X_d[b].rearrange("(p r) f -> p (r f)", p=128)
                    for c in range(NCH):
                        sync.dma_start(
                            out=xts[b * NCH + c][:, :],
                            in_=Xb[:, c * CH * FEAT : (c + 1) * CH * FEAT],
                        ).then_inc(dsems[b * NCH + c], 16)
                # output DMA after the finale
                sync.wait_ge(asem, BPC + 1)
                sync.dma_start(out=out_d[:, :], in_=res[:, :]).then_inc(osem, 16)

            @block.scalar
            def _(scalar):
                # consts on the ACT HWDGE ring: ones first (own sem so the
                # colsum is gated only on it), then wv0, wv1, wo, biases
                scalar.dma_start(out=ones[:, :], in_=ones_d[:, :]).then_inc(onesem, 16)
                scalar.dma_start(out=wv_sb[:, 0:MEM], in_=Wv_d[0]).then_inc(csem, 16)
                scalar.dma_start(out=wv_sb[:, MEM : 2 * MEM], in_=Wv_d[1]).then_inc(
                    csem, 16
                )
                scalar.dma_start(out=wo_sb[:, :], in_=Wo_d[:, :]).then_inc(csem, 16)
                scalar.dma_start(out=bias_sb[:, :], in_=bias_d[:, :]).then_inc(csem, 16)
                # per-batch psum row -> SBUF copy (transpose lhsT must be SBUF)
                for b in range(BPC):
                    scalar.wait_ge(pesem, b + 1)
                    nc.scalar.activation(
                        out=srows[b][:, :],
                        in_=pss[b][0:1, :],
                        func=AF.Copy,
                        scale=1.0,
                    ).then_inc(asem, 1)
                # final relu
                scalar.wait_ge(pesem, BPC + 4)
                scalar.wait_ge(csem, 64)
                nc.scalar.activation(
                    out=res[:, :],
                    in_=pso[:, :],
                    func=AF.Relu,
                    bias=bias_sb[:, 1:2],
                    scale=1.0,
                ).then_inc(asem, 1)

            @block.tensor
            def _(pe):
                pe.wait_ge(onesem, 16)
                for b in range(BPC):
                    k = 0
                    nmm = NCH * MM_PER_CHUNK
                    for c in range(NCH):
                        pe.wait_ge(dsems[b * NCH + c], 16)
                        for m in range(MM_PER_CHUNK):
                            ins = nc.tensor.matmul(
                                pss[b][:, :],
                                lhsT=ones[:, 0:1],
                                rhs=xts[b * NCH + c][:, m * MMW : (m + 1) * MMW],
                                start=(k == 0),
                                stop=(k == nmm - 1),
                            )
                            k += 1
                    ins.then_inc(pesem, 1)  # pesem: b+1 after batch b colsum
                # fold even/odd + transpose, per batch, via accumulating
                # PE transposes reading srows
                pe.wait_ge(vsem, 1)  # one_f memset
                for b in range(BPC):
                    pe.wait_ge(asem, b + 1)
                    last = None
                    for h in range(2):
                        nc.tensor.matmul(
                            pts[b][:, h : h + 1],
                            lhsT=srows[b][0:1, h * 128 : (h + 1) * 128],
                            rhs=one_f[0:1, 0:1],
                            is_transpose=True,
                            start=True,
                            stop=False,
                        )
                        last = nc.tensor.matmul(
                            pts[b][:, h : h + 1],
                            lhsT=srows[b][0:1, FEAT + h * 128 : FEAT + (h + 1) * 128],
                            rhs=one_f[0:1, 0:1],
                            is_transpose=True,
                            start=False,
                            stop=True,
                        )
                    last.then_inc(pesem, 1)  # pesem: BPC+1+b
                # psv = Wv'.T @ stq
                pe.wait_ge(csem, 64)
                pe.wait_ge(vsem, 1 + 2 * BPC)
                nc.tensor.matmul(
                    psv[:, :], lhsT=wv_sb[:, 0:MEM], rhs=stq[:, 0:BPC],
                    start=True, stop=False,
                )
                nc.tensor.matmul(
                    psv[:, :], lhsT=wv_sb[:, MEM : 2 * MEM], rhs=stq[:, BPC : 2 * BPC],
                    start=False, stop=True,
                ).then_inc(pesem, 1)  # pesem: BPC+3
                pe.wait_ge(vsem, 2 + 2 * BPC)  # out0 ready
                nc.tensor.matmul(
                    pso[:, :], lhsT=wo_sb[:, :], rhs=out0[:, :], start=True, stop=True
                ).then_inc(pesem, 1)  # pesem: BPC+4

            @block.vector
            def _(vector):
                nc.vector.memset(one_f[:, :], 1.0).then_inc(vsem, 1)
                # stq columns h-major: (h0b0, h0b1, h1b0, h1b1)
                for b in range(BPC):
                    vector.wait_ge(pesem, BPC + 1 + b)
                    nc.vector.tensor_copy(
                        out=stq[:, b : b + 1], in_=pts[b][:, 0:1]
                    ).then_inc(vsem, 1)
                    nc.vector.tensor_copy(
                        out=stq[:, BPC + b : BPC + b + 1], in_=pts[b][:, 1:2]
                    ).then_inc(vsem, 1)
                vector.wait_ge(pesem, BPC + 3)
                vector.wait_ge(csem, 64)
                nc.vector.tensor_scalar_add(
                    out=out0[:, :], in0=psv[:, :], scalar1=bias_sb[:, 0:1]
                ).then_inc(vsem, 1)

            @block.gpsimd
            def _(gpsimd):
                gpsimd.wait_ge(osem, 16)

            # all-engine sync, then zero the sems so a re-execution of the
            # loaded NEFF starts clean
            nc.all_engine_barrier()
            nc.gpsimd.sem_clear(range(sem_nums[0], sem_nums[-1] + 1))

    if not nc.is_finalized():
        nc.finalize()
    return nc


def kernel(X, mem, Wk, bk, Wv, bv, Wo, bo):
    global _built
    _ensure_axon_hooks()
    from concourse.bass_utils import run_bass_kernel_spmd

    if _built is None:
        _built = _build()
    nc = _built

    X = np.asarray(X, dtype=np.float32)
    Wvs = np.ascontiguousarray(
        (np.asarray(Wv, dtype=np.float32) / float(U)).reshape(2, 128, MEM)
    )
    Wos = np.ascontiguousarray(np.asarray(Wo, dtype=np.float32))
    biasc = np.zeros((MEM, 128), dtype=np.float32)
    biasc[:, 0] = np.asarray(bv, dtype=np.float32) * (N / float(U))
    biasc[:, 1] = np.asarray(bo, dtype=np.float32)
    onesc = np.ones((128, 128), dtype=np.float32)

    in_maps = [
        {
            "Xs": np.ascontiguousarray(X[i * BPC : (i + 1) * BPC]),
            "Wvs": Wvs,
            "Wos": Wos,
            "biasc": biasc,
            "onesc": onesc,
        }
        for i in range(NCORES)
    ]
    r = run_bass_kernel_spmd(nc, in_maps, list(range(NCORES)))
    kernel._last_results = r

    out = np.empty((B, MEM), dtype=np.float32)
    for i in range(NCORES):
        out[i * BPC : (i + 1) * BPC] = r.results[i]["outT"].T
    return out

